# revision 6
# baseline (speedup 1.0000x reference)
"""Trainium2 kernel for nn_MetricLogits: cdist-style metric logits.

metric[b,c] = ||w_c - f_b||^2 = ||f_b||^2 + ||w_c||^2 - 2 f_b.w_c
valuation_logits = -metric
train_logits     = -(metric - mean(metric)) = valuation_logits + mean(metric)

Sharding: weights [C,D] row-wise across 8 cores (C/8 = 6250 rows each),
feat replicated. Each core computes its [B, C/8] slice of both outputs
via a fp32r GEMM on the tensor engine plus fused bias ops on DVE.

The scalar mean(metric) is computed analytically on the host (exact
identity, no cross-core collective needed):
  mean = mean_b||f_b||^2 + mean_c||w_c||^2 - (2/(B*C)) * (sum_b f_b).(sum_c w_c)
"""
import sys

sys.path.insert(0, '/opt/trn_rl_repo')

import numpy as np

B, D, C = 4096, 1024, 50000
NCORES = 8
C_CORE = C // NCORES            # 6250
# fp32r matmul runs at 1 cyc/row only when the moving free dim >= 256,
# so split 6250 into eleven 512s + 310 + 308 (all >= 256, all even).
CHUNKS = [512] * 11 + [310, 308]
assert sum(CHUNKS) == C_CORE
KT = D // 128                   # 8 contraction tiles
BT = B // 128                   # 32 output-row tiles

_cache = {}


def _build(reps=1):
    import concourse.bacc as bacc
    import concourse.mybir as mybir
    from concourse.tile import TileContext

    nc = bacc.Bacc()
    dtr = mybir.dt.float32r
    f32 = mybir.dt.float32
    add = mybir.AluOpType.add
    sub = mybir.AluOpType.subtract

    featT2 = nc.dram_tensor("featT2", [D, B], dtr, kind="ExternalInput")
    wT = nc.dram_tensor("wT", [D, C_CORE], dtr, kind="ExternalInput")
    wsq1 = nc.dram_tensor("wsq1", [128, C_CORE], f32, kind="ExternalInput")
    negfsq = nc.dram_tensor("negfsq", [128, BT], f32, kind="ExternalInput")
    meanin = nc.dram_tensor("meanin", [128, 1], f32, kind="ExternalInput")
    val = nc.dram_tensor("val", [B, C_CORE], f32, kind="ExternalOutput")
    train = nc.dram_tensor("train", [B, C_CORE], f32, kind="ExternalOutput")

    featT2_r = featT2.rearrange("(k p) b -> p k b", p=128)   # [128, KT, B]
    wT_r = wT.rearrange("(k p) c -> p k c", p=128)           # [128, KT, C_CORE]

    with TileContext(nc) as tc:
        with tc.tile_pool(name="const", bufs=1) as constp, \
             tc.tile_pool(name="wchunk", bufs=2) as wp, \
             tc.tile_pool(name="sqchunk", bufs=2) as sqp, \
             tc.tile_pool(name="outs", bufs=5) as outp, \
             tc.tile_pool(name="ps", bufs=8, space="PSUM") as ps:
            feat_sb = constp.tile([128, KT, B], dtr)
            for k in range(KT):
                nc.sync.dma_start(out=feat_sb[:, k, :], in_=featT2_r[:, k, :])
            nfsq_sb = constp.tile([128, BT], f32)
            nc.sync.dma_start(out=nfsq_sb[:], in_=negfsq[:, :])
            mean_sb = constp.tile([128, 1], f32)
            nc.sync.dma_start(out=mean_sb[:], in_=meanin[:, :])

            for _rep in range(reps):
                c0 = 0
                for cn in CHUNKS:
                    w_sb = wp.tile([128, KT, 512], dtr, tag="w")
                    nc.sync.dma_start(out=w_sb[:, :, :cn], in_=wT_r[:, :, c0:c0 + cn])
                    sq_sb = sqp.tile([128, 512], f32, tag="sq")
                    nc.sync.dma_start(out=sq_sb[:, :cn], in_=wsq1[:, c0:c0 + cn])
                    for b in range(BT):
                        psum = ps.tile([128, 512], f32, tag="psum")
                        for k in range(KT):
                            nc.tensor.matmul(psum[:, :cn],
                                             feat_sb[:, k, b * 128:(b + 1) * 128],
                                             w_sb[:, k, :cn],
                                             start=(k == 0), stop=(k == KT - 1))
                        vt = outp.tile([128, 512], f32, tag="val")
                        tt = outp.tile([128, 512], f32, tag="train")
                        # val = (2*cross - fsq) - wsq ; train = val + mean
                        nc.vector.scalar_tensor_tensor(
                            out=vt[:, :cn], in0=psum[:, :cn],
                            scalar=nfsq_sb[:, b:b + 1], in1=sq_sb[:, :cn],
                            op0=add, op1=sub)
                        nc.vector.tensor_scalar_add(tt[:, :cn], vt[:, :cn],
                                                    mean_sb[:, 0:1])
                        nc.sync.dma_start(out=val[b * 128:(b + 1) * 128, c0:c0 + cn],
                                          in_=vt[:, :cn])
                        nc.sync.dma_start(out=train[b * 128:(b + 1) * 128, c0:c0 + cn],
                                          in_=tt[:, :cn])
                    c0 += cn
    nc.finalize()
    return nc


def _get_nc():
    if 'nc' not in _cache:
        _cache['nc'] = _build()
    return _cache['nc']


def kernel(feat, label, weights):
    from concourse.bass_utils import run_bass_kernel_spmd

    feat = np.asarray(feat, dtype=np.float32)
    weights = np.asarray(weights, dtype=np.float32)

    # Host-side (cheap, O((B+C)D)) exact-identity prep in float64.
    feat64 = feat.astype(np.float64)
    w64 = weights.astype(np.float64)
    fsq = np.einsum('bd,bd->b', feat64, feat64)          # [B]
    wsq = np.einsum('cd,cd->c', w64, w64)                # [C]
    mean_metric = (fsq.mean() + wsq.mean()
                   - 2.0 / (B * C) * (feat64.sum(0) @ w64.sum(0)))

    featT2 = np.ascontiguousarray((2.0 * feat).T)                  # [D, B] f32
    negfsq = np.ascontiguousarray((-fsq).astype(np.float32)
                                  .reshape(BT, 128).T)             # [128, BT]

    in_maps = []
    for i in range(NCORES):
        sl = slice(i * C_CORE, (i + 1) * C_CORE)
        wT = np.ascontiguousarray(weights[sl].T)                   # [D, C_CORE]
        wsq_i = wsq[sl].astype(np.float32)                         # [C_CORE]
        wsq1 = np.broadcast_to(wsq_i[None, :], (128, C_CORE)).copy()
        in_maps.append({
            "featT2": featT2,
            "wT": wT,
            "wsq1": wsq1,
            "negfsq": negfsq,
            "meanin": np.full((128, 1), mean_metric, np.float32),
        })

    res = run_bass_kernel_spmd(_get_nc(), in_maps, core_ids=list(range(NCORES)))

    valuation = np.concatenate([r["val"] for r in res.results], axis=1)
    train = np.concatenate([r["train"] for r in res.results], axis=1)
    return (valuation, train, weights)



# revision 8
# speedup vs baseline: 1.1857x; 1.1857x over previous
"""Trainium2 kernel for nn_MetricLogits: cdist-style metric logits.

metric[b,c] = ||w_c - f_b||^2 = ||f_b||^2 + ||w_c||^2 - 2 f_b.w_c
valuation_logits = -metric
train_logits     = -(metric - mean(metric)) = valuation_logits + mean(metric)

Sharding: weights [C,D] row-wise across 8 cores (C/8 = 6250 rows each),
feat replicated. Each core computes its [B, C/8] slice of both outputs
via a fp32r GEMM on the tensor engine plus fused bias ops on DVE.

The scalar mean(metric) is computed analytically on the host (exact
identity, no cross-core collective needed):
  mean = mean_b||f_b||^2 + mean_c||w_c||^2 - (2/(B*C)) * (sum_b f_b).(sum_c w_c)
"""
import sys

sys.path.insert(0, '/opt/trn_rl_repo')

import numpy as np

B, D, C = 4096, 1024, 50000
NCORES = 8
C_CORE = C // NCORES            # 6250
# Output tiles are 1250 columns wide (5 groups/core) so each output DMA
# moves 128x1250 f32 with 5KB-contiguous descriptors. Each group is computed
# as 3 matmul sub-chunks; fp32r needs the moving free dim even and >= 256
# for the 1 cyc/row rate, hence 418+416+416.
WGROUPS = [(g0, [418, 416, 416]) for g0 in range(0, C_CORE, 1250)]
WIDE = 1250
KT = D // 128                   # 8 contraction tiles
BT = B // 128                   # 32 output-row tiles

_cache = {}


def _build(reps=1):
    import concourse.bacc as bacc
    import concourse.mybir as mybir
    from concourse.tile import TileContext

    nc = bacc.Bacc()
    dtr = mybir.dt.float32r
    f32 = mybir.dt.float32
    add = mybir.AluOpType.add
    sub = mybir.AluOpType.subtract

    featT2 = nc.dram_tensor("featT2", [D, B], dtr, kind="ExternalInput")
    wT = nc.dram_tensor("wT", [D, C_CORE], dtr, kind="ExternalInput")
    wsq1 = nc.dram_tensor("wsq1", [128, C_CORE], f32, kind="ExternalInput")
    negfsq = nc.dram_tensor("negfsq", [128, BT], f32, kind="ExternalInput")
    meanin = nc.dram_tensor("meanin", [128, 1], f32, kind="ExternalInput")
    val = nc.dram_tensor("val", [B, C_CORE], f32, kind="ExternalOutput")
    train = nc.dram_tensor("train", [B, C_CORE], f32, kind="ExternalOutput")

    featT2_r = featT2.rearrange("(k p) b -> p k b", p=128)   # [128, KT, B]
    wT_r = wT.rearrange("(k p) c -> p k c", p=128)           # [128, KT, C_CORE]

    with TileContext(nc) as tc:
        with tc.tile_pool(name="const", bufs=1) as constp, \
             tc.tile_pool(name="wchunk", bufs=3) as wp, \
             tc.tile_pool(name="sqchunk", bufs=1) as sqp, \
             tc.tile_pool(name="outs", bufs=2) as outp, \
             tc.tile_pool(name="ps", bufs=8, space="PSUM") as ps:
            feat_sb = constp.tile([128, KT, B], dtr)
            for k in range(KT):
                nc.sync.dma_start(out=feat_sb[:, k, :], in_=featT2_r[:, k, :])
            nfsq_sb = constp.tile([128, BT], f32)
            nc.sync.dma_start(out=nfsq_sb[:], in_=negfsq[:, :])
            mean_sb = constp.tile([128, 1], f32)
            nc.sync.dma_start(out=mean_sb[:], in_=meanin[:, :])

            for _rep in range(reps):
                for g0, subs in WGROUPS:
                    w_tiles = []
                    off = 0
                    for cn in subs:
                        w_sb = wp.tile([128, KT, 512], dtr, tag="w")
                        nc.sync.dma_start(out=w_sb[:, :, :cn],
                                          in_=wT_r[:, :, g0 + off:g0 + off + cn])
                        w_tiles.append((off, cn, w_sb))
                        off += cn
                    sq_sb = sqp.tile([128, WIDE], f32, tag="sq")
                    nc.sync.dma_start(out=sq_sb[:, :], in_=wsq1[:, g0:g0 + WIDE])
                    for b in range(BT):
                        vt = outp.tile([128, WIDE], f32, tag="val")
                        tt = outp.tile([128, WIDE], f32, tag="train")
                        for off, cn, w_sb in w_tiles:
                            psum = ps.tile([128, 512], f32, tag="psum")
                            for k in range(KT):
                                nc.tensor.matmul(psum[:, :cn],
                                                 feat_sb[:, k, b * 128:(b + 1) * 128],
                                                 w_sb[:, k, :cn],
                                                 start=(k == 0), stop=(k == KT - 1))
                            # val = (2*cross - fsq) - wsq ; train = val + mean
                            nc.vector.scalar_tensor_tensor(
                                out=vt[:, off:off + cn], in0=psum[:, :cn],
                                scalar=nfsq_sb[:, b:b + 1], in1=sq_sb[:, off:off + cn],
                                op0=add, op1=sub)
                            nc.vector.tensor_scalar_add(tt[:, off:off + cn],
                                                        vt[:, off:off + cn],
                                                        mean_sb[:, 0:1])
                        nc.sync.dma_start(out=val[b * 128:(b + 1) * 128, g0:g0 + WIDE],
                                          in_=vt[:, :])
                        nc.sync.dma_start(out=train[b * 128:(b + 1) * 128, g0:g0 + WIDE],
                                          in_=tt[:, :])
    nc.finalize()
    return nc


def _get_nc():
    if 'nc' not in _cache:
        _cache['nc'] = _build()
    return _cache['nc']


def kernel(feat, label, weights):
    from concourse.bass_utils import run_bass_kernel_spmd

    feat = np.asarray(feat, dtype=np.float32)
    weights = np.asarray(weights, dtype=np.float32)

    # Host-side (cheap, O((B+C)D)) exact-identity prep in float64.
    feat64 = feat.astype(np.float64)
    w64 = weights.astype(np.float64)
    fsq = np.einsum('bd,bd->b', feat64, feat64)          # [B]
    wsq = np.einsum('cd,cd->c', w64, w64)                # [C]
    mean_metric = (fsq.mean() + wsq.mean()
                   - 2.0 / (B * C) * (feat64.sum(0) @ w64.sum(0)))

    featT2 = np.ascontiguousarray((2.0 * feat).T)                  # [D, B] f32
    negfsq = np.ascontiguousarray((-fsq).astype(np.float32)
                                  .reshape(BT, 128).T)             # [128, BT]

    in_maps = []
    for i in range(NCORES):
        sl = slice(i * C_CORE, (i + 1) * C_CORE)
        wT = np.ascontiguousarray(weights[sl].T)                   # [D, C_CORE]
        wsq_i = wsq[sl].astype(np.float32)                         # [C_CORE]
        wsq1 = np.broadcast_to(wsq_i[None, :], (128, C_CORE)).copy()
        in_maps.append({
            "featT2": featT2,
            "wT": wT,
            "wsq1": wsq1,
            "negfsq": negfsq,
            "meanin": np.full((128, 1), mean_metric, np.float32),
        })

    res = run_bass_kernel_spmd(_get_nc(), in_maps, core_ids=list(range(NCORES)))

    valuation = np.concatenate([r["val"] for r in res.results], axis=1)
    train = np.concatenate([r["train"] for r in res.results], axis=1)
    return (valuation, train, weights)

